# revision 1
# baseline (speedup 1.0000x reference)
"""CFConv (SchNet-style continuous-filter convolution) Bass kernel for 8 trn2 cores.

Computation:  f = x@W1;  wf = w_ij * f[idx_j];  conv = segment_sum(wf, seg_i);
              out = conv@W2 + b2

Sharding: edges split equally across 8 cores at segment boundaries. Each core
computes the full node-feature table f = x@W1 (replicated), gathers neighbor
rows with dma_gather, multiplies by w_ij, segment-sums via one-hot matmuls on
the PE (PSUM accumulation over 128-atom windows), applies W2, and writes
per-window partial outputs. Host overlap-adds window outputs (exact: @W2 is
linear) and adds b2.

dma_gather indices are int16, so the f table is split at row 32768 into two
tables; each core's edges are processed in two phases (A: idx<32768, B: rest).
Segment-sum linearity makes the phase split exact.
"""

import math
import os
import sys

import numpy as np

for _p in ("/opt/trn_rl_repo", "/root/.axon_site/_ro/trn_rl_repo"):
    if os.path.isdir(_p) and _p not in sys.path:
        sys.path.insert(0, _p)

import ml_dtypes

BF16 = ml_dtypes.bfloat16

# Problem shape (hardcoded per harness contract)
N_ATOMS = 50000
N_EDGES = 1600000
D = 128
N_CORES = 8

TBL_SPLIT = 32768  # int16 gather-index limit

# Block geometry: GPW groups of 128 edges per PSUM window, WPB windows per block
GP = 128  # edges per group (matmul contraction dim)
WPB = 4  # windows per block (PSUM bank = 4*128 fp32 columns)


def _pick_gpw(spans_ok):
    # spans_ok(gpw) -> bool; prefer big blocks
    for gpw in (7, 4, 2, 1):
        if spans_ok(gpw):
            return gpw
    raise ValueError("cannot window edges: segment spans too wide even at gpw=1")


def _core_edge_cuts(seg):
    """Split edges into N_CORES ranges at segment boundaries, near-equal sizes."""
    E = len(seg)
    cuts = [0]
    for k in range(1, N_CORES):
        t = k * E // N_CORES
        # move cut to the first edge of seg[t]'s atom
        a = seg[t]
        cut = int(np.searchsorted(seg, a, side="left"))
        cuts.append(max(cut, cuts[-1]))
    cuts.append(E)
    return cuts


def _prep_phase(w, idx_local, seg, gpw):
    """Build device arrays for one (core, phase) edge list.

    w: [n,128] float32 edge filters, idx_local: [n] int64 table-local gather
    rows, seg: [n] int64 global atom ids (sorted). Returns dict with per-block
    tiled arrays, or None if a window span exceeds 128.
    """
    groups = gpw * WPB
    blk = groups * GP
    n = len(seg)
    nblk = max(1, math.ceil(n / blk))
    npad = nblk * blk

    w_pad = np.zeros((npad, D), dtype=np.float32)
    w_pad[:n] = w
    idx_pad = np.zeros(npad, dtype=np.int64)
    idx_pad[:n] = idx_local
    seg_pad = np.zeros(npad, dtype=np.int64)
    seg_pad[:n] = seg

    # window bases + local atom ids
    win_edges = gpw * GP
    nwin = nblk * WPB
    seg_w = seg_pad.reshape(nwin, win_edges)
    # base = seg of first real edge in window; fully-padded windows -> base 0
    bases = seg_w[:, 0].copy()
    n_full_wins = n // win_edges
    if n_full_wins < nwin and n % win_edges == 0 and n > 0:
        pass  # seg_w[n_full_wins][0] is already a pad (0)
    # pad tail of the partial window: give pads the window's base so c=0
    if n < npad:
        w_first = n // win_edges
        base_partial = seg_pad[w_first * win_edges] if n % win_edges else 0
        if n % win_edges:
            bases[w_first] = base_partial
            seg_pad[n : (w_first + 1) * win_edges] = base_partial
        # fully-padded windows already have seg=0, base=0
    c = seg_pad - np.repeat(bases, win_edges)
    cmax = int(c.max()) if npad else 0
    if cmax >= 128 or c.min() < 0:
        return None

    # tile layouts
    # edge i of block at [i%128 partition, i//128 group]
    wt = (
        w_pad.astype(BF16)
        .reshape(nblk, groups, GP, D)
        .transpose(0, 2, 1, 3)
        .copy()
    )  # [nblk, 128, groups, 128]
    ct = c.astype(np.float32).reshape(nblk, groups, GP).transpose(0, 2, 1).copy()
    # idx wrapped: position i = s*16 + p -> [p, s]
    it = (
        idx_pad.astype(np.int16)
        .reshape(nblk, blk // 16, 16)
        .transpose(0, 2, 1)
    )  # [nblk, 16, blk//16]
    it = np.tile(it, (1, 8, 1)).copy()  # replicate to 128 partitions
    bases = bases.reshape(nblk, WPB)
    return dict(wt=wt, ct=ct, it=it, bases=bases, nblk=nblk)


def _zero_blocks(nblk, gpw):
    groups = gpw * WPB
    blk = groups * GP
    return dict(
        wt=np.zeros((nblk, GP, groups, D), dtype=BF16),
        ct=np.zeros((nblk, GP, groups), dtype=np.float32),
        it=np.zeros((nblk, 128, blk // 16), dtype=np.int16),
        bases=np.zeros((nblk, WPB), dtype=np.int64),
        nblk=nblk,
    )


def _pad_blocks(ph, nblk, gpw):
    if ph["nblk"] == nblk:
        return ph
    z = _zero_blocks(nblk - ph["nblk"], gpw)
    return dict(
        wt=np.concatenate([ph["wt"], z["wt"]]),
        ct=np.concatenate([ph["ct"], z["ct"]]),
        it=np.concatenate([ph["it"], z["it"]]),
        bases=np.concatenate([ph["bases"], z["bases"]]),
        nblk=nblk,
    )


def prep_inputs(x, w_ij, seg_i, idx_j, W1, W2):
    """Host-side preparation. Returns (per_core_maps, shared, plan)."""
    seg = np.asarray(seg_i, dtype=np.int64)
    idx = np.asarray(idx_j, dtype=np.int64)
    w = np.asarray(w_ij, dtype=np.float32)
    x = np.asarray(x, dtype=np.float32)

    cuts = _core_edge_cuts(seg)

    def spans_ok(gpw):
        win_edges = gpw * GP * 1
        # check all (core, phase) windows
        for k in range(N_CORES):
            lo, hi = cuts[k], cuts[k + 1]
            m = idx[lo:hi] < TBL_SPLIT
            for sel in (m, ~m):
                s = seg[lo:hi][sel]
                nw = math.ceil(len(s) / (gpw * GP))
                for wi in range(nw):
                    ss = s[wi * gpw * GP : (wi + 1) * gpw * GP]
                    if len(ss) and ss[-1] - ss[0] >= 128:
                        return False
        return True

    gpw = _pick_gpw(spans_ok)
    groups = gpw * WPB

    phases = []  # [core][phase] dicts
    for k in range(N_CORES):
        lo, hi = cuts[k], cuts[k + 1]
        m = idx[lo:hi] < TBL_SPLIT
        pair = []
        for pi, sel in enumerate((m, ~m)):
            e = np.nonzero(sel)[0] + lo
            ph = _prep_phase(
                w[e],
                idx[e] - (0 if pi == 0 else TBL_SPLIT),
                seg[e],
                gpw,
            )
            assert ph is not None, "span check passed but prep failed"
            pair.append(ph)
        phases.append(pair)

    nblk_a = max(p[0]["nblk"] for p in phases)
    nblk_b = max(p[1]["nblk"] for p in phases)
    per_core = []
    all_bases = []
    for k in range(N_CORES):
        pa = _pad_blocks(phases[k][0], nblk_a, gpw)
        pb = _pad_blocks(phases[k][1], nblk_b, gpw)
        per_core.append(
            dict(
                wt=np.concatenate([pa["wt"], pb["wt"]]),
                ct=np.concatenate([pa["ct"], pb["ct"]]),
                it=np.concatenate([pa["it"], pb["it"]]),
            )
        )
        all_bases.append(np.concatenate([pa["bases"], pb["bases"]]))

    # shared tensors
    n_atoms_pad = math.ceil(N_ATOMS / 512) * 512
    xT = np.zeros((D, n_atoms_pad), dtype=BF16)
    xT[:, :N_ATOMS] = x.T.astype(BF16)
    iota = np.broadcast_to(np.arange(GP, dtype=np.float32), (GP, GP)).astype(BF16)
    shared = dict(
        xT=np.ascontiguousarray(xT),
        W1=W1.astype(BF16),
        W2=W2.astype(np.float32),
        iota=np.ascontiguousarray(iota),
    )
    plan = dict(
        gpw=gpw,
        groups=groups,
        nblk_a=nblk_a,
        nblk_b=nblk_b,
        nblk=nblk_a + nblk_b,
        n_atoms_pad=n_atoms_pad,
        bases=all_bases,
    )
    return per_core, shared, plan


def host_combine(stages, plan, b2):
    """stages: list of [NBLK, 128, WPB*128] fp32 outT arrays (per core)."""
    out = np.zeros((N_ATOMS + GP, D), dtype=np.float64)
    for k in range(N_CORES):
        st = stages[k].astype(np.float64)
        nblk = plan["nblk"]
        # [NBLK, 128do, WPB, 128a] -> [NBLK, WPB, 128a, 128do]
        st = st.reshape(nblk, D, WPB, GP).transpose(0, 2, 3, 1)
        bases = plan["bases"][k]
        for b in range(nblk):
            for wi in range(WPB):
                base = int(bases[b, wi])
                out[base : base + GP] += st[b, wi]
    return (out[:N_ATOMS] + np.asarray(b2, dtype=np.float64)).astype(np.float32)


# ---------------------------------------------------------------------------
# numpy emulation of the device program (for validating the decomposition)
# ---------------------------------------------------------------------------


def emulate_device(per_core, shared, plan, exact=False):
    cast = (lambda a: a.astype(np.float32)) if exact else (
        lambda a: a.astype(BF16).astype(np.float32)
    )
    xT = shared["xT"].astype(np.float32)
    W1 = shared["W1"].astype(np.float32)
    W2 = shared["W2"].astype(np.float32)
    f = cast(xT.T @ W1)  # [n_atoms_pad, 128] (bf16-rounded)
    groups = plan["groups"]
    gpw = plan["gpw"]
    stages = []
    for k in range(N_CORES):
        m = per_core[k]
        nblk = plan["nblk"]
        stage = np.zeros((nblk, D, WPB * GP), dtype=np.float32)
        for b in range(nblk):
            tbl_off = 0 if b < plan["nblk_a"] else TBL_SPLIT
            idx = m["it"][b][:16].T.reshape(-1).astype(np.int64)  # [blk] in (s p) order
            w_t = m["wt"][b].astype(np.float32)  # [128, groups, 128]
            c_t = m["ct"][b].astype(np.float32)  # [128, groups]
            fj = f[idx + tbl_off].reshape(groups, GP, D).transpose(1, 0, 2)
            wf = cast(w_t * fj)  # [128, groups, 128]
            convT = np.zeros((D, WPB * GP), dtype=np.float32)
            for g in range(groups):
                S = (c_t[:, g : g + 1] == np.arange(GP)[None, :]).astype(np.float32)
                wi = g // gpw
                convT[:, wi * GP : (wi + 1) * GP] += wf[:, g, :].T @ S
            stage[b] = W2.T @ convT
        stages.append(stage)
    return stages


# ---------------------------------------------------------------------------
# bass device program
# ---------------------------------------------------------------------------


def build_program(plan):
    import concourse.bacc as bacc
    import concourse.mybir as mybir
    import concourse.tile as tile

    fp32 = mybir.dt.float32
    bf16 = mybir.dt.bfloat16
    i16 = mybir.dt.int16

    groups = plan["groups"]
    gpw = plan["gpw"]
    blk = groups * GP
    nblk = plan["nblk"]
    nblk_a = plan["nblk_a"]
    nap = plan["n_atoms_pad"]

    nc = bacc.Bacc(
        "TRN2",
        target_bir_lowering=False,
        debug=False,
        num_devices=N_CORES,
    )

    xT_d = nc.dram_tensor("xT", [D, nap], bf16, kind="ExternalInput")
    W1_d = nc.dram_tensor("W1", [D, D], bf16, kind="ExternalInput")
    W2_d = nc.dram_tensor("W2", [D, D], fp32, kind="ExternalInput")
    iota_d = nc.dram_tensor("iota", [GP, GP], bf16, kind="ExternalInput")
    wt_d = nc.dram_tensor("wt", [nblk, GP, groups, D], bf16, kind="ExternalInput")
    ct_d = nc.dram_tensor("ct", [nblk, GP, groups], fp32, kind="ExternalInput")
    it_d = nc.dram_tensor("it", [nblk, 128, blk // 16], i16, kind="ExternalInput")
    stage_d = nc.dram_tensor(
        "stage", [nblk, D, WPB * GP], fp32, kind="ExternalOutput"
    )

    with tile.TileContext(nc) as tc:
        with (
            tc.tile_pool(name="consts", bufs=1) as consts,
            tc.tile_pool(name="dram", bufs=1, space="DRAM") as dram_pool,
        ):
            f_d = dram_pool.tile([nap, D], bf16)

            W1_sb = consts.tile([D, D], bf16)
            nc.sync.dma_start(W1_sb[:], W1_d[:])
            W2_sb = consts.tile([D, D], fp32)
            nc.sync.dma_start(W2_sb[:], W2_d[:])
            iota_sb = consts.tile([GP, GP], bf16)
            nc.sync.dma_start(iota_sb[:], iota_d[:])

            # ---------------- f-phase: f = x @ W1 ----------------
            CH = 4  # x tiles per chunk
            nchunks = nap // (CH * GP)
            with (
                tc.tile_pool(name="xt", bufs=3) as xt_pool,
                tc.tile_pool(name="fsb", bufs=3) as fsb_pool,
                tc.tile_pool(name="fps", bufs=4, space="PSUM") as fps_pool,
            ):
                for ci in range(nchunks):
                    a0 = ci * CH * GP
                    xt = xt_pool.tile([D, CH * GP], bf16)
                    nc.sync.dma_start(xt[:], xT_d[:, a0 : a0 + CH * GP])
                    fsb = fsb_pool.tile([GP, CH, D], bf16)
                    for i in range(CH):
                        fps = fps_pool.tile([GP, D], fp32)
                        nc.tensor.matmul(
                            fps[:],
                            xt[:, i * GP : (i + 1) * GP],
                            W1_sb[:],
                            start=True,
                            stop=True,
                        )
                        nc.scalar.copy(fsb[:, i, :], fps[:])
                    nc.sync.dma_start(
                        f_d[a0 : a0 + CH * GP, :].rearrange(
                            "(i p) d -> p i d", p=GP
                        ),
                        fsb[:],
                    )

            # ---------------- main loop ----------------
            with (
                tc.tile_pool(name="wsb", bufs=3) as w_pool,
                tc.tile_pool(name="fj", bufs=3) as fj_pool,
                tc.tile_pool(name="wf", bufs=2) as wf_pool,
                tc.tile_pool(name="S", bufs=2) as s_pool,
                tc.tile_pool(name="ct", bufs=3) as ct_pool,
                tc.tile_pool(name="it", bufs=3) as it_pool,
                tc.tile_pool(name="cvs", bufs=2) as cvs_pool,
                tc.tile_pool(name="os", bufs=2) as os_pool,
                tc.tile_pool(name="cvp", bufs=2, space="PSUM") as cvp_pool,
                tc.tile_pool(name="otp", bufs=2, space="PSUM") as otp_pool,
            ):
                for b in range(nblk):
                    w_sb = w_pool.tile([GP, groups, D], bf16)
                    nc.sync.dma_start(w_sb[:], wt_d[b])
                    ct_sb = ct_pool.tile([GP, groups], fp32)
                    nc.sync.dma_start(ct_sb[:], ct_d[b])
                    it_sb = it_pool.tile([128, blk // 16], i16)
                    nc.sync.dma_start(it_sb[:], it_d[b])

                    fj_sb = fj_pool.tile([GP, groups, D], bf16)
                    tbl = (
                        f_d[0:TBL_SPLIT, :]
                        if b < nblk_a
                        else f_d[TBL_SPLIT:nap, :]
                    )
                    nc.gpsimd.dma_gather(
                        fj_sb[:],
                        tbl,
                        it_sb[:],
                        blk,
                        blk,
                        D,
                        single_packet=False,
                    )

                    wf_sb = wf_pool.tile([GP, groups, D], bf16)
                    nc.vector.tensor_mul(wf_sb[:], w_sb[:], fj_sb[:])

                    s_sb = s_pool.tile([GP, groups, D], bf16)
                    for g in range(groups):
                        nc.vector.tensor_scalar(
                            s_sb[:, g, :],
                            iota_sb[:],
                            ct_sb[:, g : g + 1],
                            None,
                            mybir.AluOpType.is_equal,
                        )

                    cvp = cvp_pool.tile([D, WPB, GP], fp32)
                    for g in range(groups):
                        wi = g // gpw
                        nc.tensor.matmul(
                            cvp[:, wi, :],
                            wf_sb[:, g, :],
                            s_sb[:, g, :],
                            start=(g % gpw == 0),
                            stop=(g % gpw == gpw - 1),
                        )
                    cvs = cvs_pool.tile([D, WPB * GP], fp32)
                    nc.scalar.copy(cvs[:], cvp[:].rearrange("d w a -> d (w a)"))

                    otp = otp_pool.tile([D, WPB * GP], fp32)
                    nc.tensor.matmul(
                        otp[:], W2_sb[:], cvs[:], start=True, stop=True
                    )
                    osb = os_pool.tile([D, WPB * GP], fp32)
                    nc.scalar.copy(osb[:], otp[:])
                    nc.sync.dma_start(stage_d[b], osb[:])

    nc.compile()
    return nc


def run_device(per_core, shared, plan, trace=False):
    from concourse import bass_utils

    nc = build_program(plan)
    in_maps = []
    for k in range(N_CORES):
        m = dict(shared)
        m.update(per_core[k])
        in_maps.append(
            {
                "xT": np.ascontiguousarray(m["xT"]),
                "W1": np.ascontiguousarray(m["W1"]),
                "W2": np.ascontiguousarray(m["W2"]),
                "iota": np.ascontiguousarray(m["iota"]),
                "wt": np.ascontiguousarray(m["wt"]),
                "ct": np.ascontiguousarray(m["ct"]),
                "it": np.ascontiguousarray(m["it"]),
            }
        )
    res = bass_utils.run_bass_kernel_spmd(
        nc, in_maps, core_ids=list(range(N_CORES)), trace=trace
    )
    stages = [r["stage"] for r in res.results]
    return stages, res


def bench_device(per_core, shared, plan, iters=24):
    """Steady-state per-execution device time via async repeat dispatch."""
    import time

    import jax
    from jax.sharding import Mesh, PartitionSpec
    from jax.experimental.shard_map import shard_map
    import concourse.bass2jax as bass2jax
    from concourse.bass2jax import (
        _bass_exec_p,
        install_neuronx_cc_hook,
        partition_id_tensor,
    )
    import concourse.mybir as mybir

    install_neuronx_cc_hook()
    nc = build_program(plan)
    partition_name = (
        nc.partition_id_tensor.name if nc.partition_id_tensor else None
    )

    in_names = []
    out_names = []
    out_avals = []
    zero_outs = []
    for alloc in nc.m.functions[0].allocations:
        if not isinstance(alloc, mybir.MemoryLocationSet):
            continue
        name = alloc.memorylocations[0].name
        if alloc.kind == "ExternalInput":
            if name != partition_name:
                in_names.append(name)
        elif alloc.kind == "ExternalOutput":
            out_names.append(name)
            dt = mybir.dt.np(alloc.dtype)
            out_avals.append(
                jax.core.ShapedArray(tuple(alloc.tensor_shape), dt)
            )
            zero_outs.append(np.zeros(tuple(alloc.tensor_shape), dt))
    n_params = len(in_names)
    all_names = in_names + out_names
    if partition_name is not None:
        all_names = all_names + [partition_name]

    def _body(*args):
        operands = list(args)
        if partition_name is not None:
            operands.append(partition_id_tensor())
        outs = _bass_exec_p.bind(
            *operands,
            out_avals=tuple(out_avals),
            in_names=tuple(all_names),
            out_names=tuple(out_names),
            lowering_input_output_aliases=(),
            sim_require_finite=True,
            sim_require_nnan=True,
            nc=nc,
        )
        return tuple(outs)

    devices = jax.devices()[:N_CORES]
    mesh = Mesh(np.asarray(devices), ("core",))
    nin = n_params + len(zero_outs)
    sharded = jax.jit(
        shard_map(
            _body,
            mesh=mesh,
            in_specs=(PartitionSpec("core"),) * nin,
            out_specs=(PartitionSpec("core"),) * len(out_names),
            check_rep=False,
        ),
        keep_unused=True,
    )

    in_maps = []
    for k in range(N_CORES):
        m = dict(shared)
        m.update(per_core[k])
        in_maps.append(m)
    concat = [
        np.concatenate([np.asarray(in_maps[c][n]) for c in range(N_CORES)], axis=0)
        for n in in_names
    ] + [np.zeros((N_CORES * z.shape[0], *z.shape[1:]), z.dtype) for z in zero_outs]
    from jax.sharding import NamedSharding

    sh = NamedSharding(mesh, PartitionSpec("core"))
    dev_in = [jax.device_put(a, sh) for a in concat]

    # warmup (compile + first run)
    out = sharded(*dev_in)
    jax.block_until_ready(out)
    t0 = time.perf_counter()
    out = sharded(*dev_in)
    jax.block_until_ready(out)
    t1 = time.perf_counter()
    single = t1 - t0
    # min-of-batches steady state (robust to terminal noise)
    batch = 8
    nbatches = max(1, iters // batch)
    times = []
    for _ in range(nbatches):
        tb = time.perf_counter()
        outs = [sharded(*dev_in) for _ in range(batch)]
        jax.block_until_ready(outs)
        times.append((time.perf_counter() - tb) / batch)
    per_iter = min(times)
    stage_g = np.asarray(out[0]).reshape(N_CORES, *out_avals[0].shape)
    stages = [stage_g[c] for c in range(N_CORES)]
    return stages, dict(single_s=single, per_iter_s=per_iter)


def kernel(x, w_ij, seg_i, idx_j, seg_i_sum, W1, W2, b2, _trace=False, _emulate=False):
    per_core, shared, plan = prep_inputs(x, w_ij, seg_i, idx_j, W1, W2)
    if _emulate:
        stages = emulate_device(per_core, shared, plan)
        res = None
    else:
        stages, res = run_device(per_core, shared, plan, trace=_trace)
    out = host_combine(stages, plan, b2)
    if _trace:
        return out, res
    return out



# revision 4
# speedup vs baseline: 1.6566x; 1.6566x over previous
"""CFConv Bass kernel v2 for 8 trn2 cores.

f = x@W1 ; wf = w_ij * f[idx_j] ; conv = segment_sum(wf, seg_i) ; out = conv@W2 + b2

Design vs v1 baseline:
- Atom space tiled into 392 aligned blocks of 128; cores own disjoint
  contiguous block ranges (no cross-core overlap; host combine is concat).
- Edges grouped per (window of 4 blocks, quarter, phase) with uniform group
  counts across cores/windows so the SPMD program is static.
- Gather: DRAM-source dma_gather split across 4 SWDGE queues (overlaps the
  random-row HBM latency; measured ~2.1x faster than single queue).
- Segment-sum via one-hot matmuls accumulated in PSUM per atom block; W2
  applied on device per window; output is disjoint per core.
"""

import math
import os
import sys
import time

import numpy as np

for _p in ("/opt/trn_rl_repo", "/root/.axon_site/_ro/trn_rl_repo"):
    if os.path.isdir(_p) and _p not in sys.path:
        sys.path.insert(0, _p)

import ml_dtypes

BF16 = ml_dtypes.bfloat16

N_ATOMS = 50000
N_EDGES = 1600000
D = 128
N_CORES = 8
GP = 128
NAP = 50176          # padded atoms (392 blocks of 128)
NBLK = NAP // GP     # 392
TBL_SPLIT = 32768    # int16 gather-index limit
CHUNK_MAX = 48       # max groups per gather call / SBUF chunk


def prep_inputs(x, w_ij, seg_i, idx_j, W1, W2):
    seg = np.asarray(seg_i).astype(np.int64)
    idx = np.asarray(idx_j).astype(np.int64)
    w = np.asarray(w_ij, dtype=np.float32)
    x = np.asarray(x, dtype=np.float32)
    E = len(seg)

    # edge offsets at atom-block boundaries
    bs = np.searchsorted(seg, np.arange(0, NBLK * GP + 1, GP))  # [NBLK+1]

    # core cuts at block granularity, balanced by edge count
    cuts = [0]
    for k in range(1, N_CORES):
        tgt = k * E // N_CORES
        b = int(np.searchsorted(bs, tgt))
        b = max(min(b, NBLK - (N_CORES - k)), cuts[-1] + 1)
        if abs(int(bs[b - 1]) - tgt) < abs(int(bs[b]) - tgt):
            b = max(b - 1, cuts[-1] + 1)
        cuts.append(b)
    cuts.append(NBLK)

    NW = max(math.ceil((cuts[k + 1] - cuts[k]) / 4) for k in range(N_CORES))

    isA = idx < TBL_SPLIT

    # per (core, window, quarter, phase) group counts -> global maxes
    def block_phase_edges(t):
        lo, hi = int(bs[t]), int(bs[t + 1])
        m = isA[lo:hi]
        return lo + np.nonzero(m)[0], lo + np.nonzero(~m)[0]

    GA = GB = 1
    per_block = {}
    for t in range(NBLK):
        ea, eb = block_phase_edges(t)
        per_block[t] = (ea, eb)
        GA = max(GA, math.ceil(len(ea) / GP) or 1)
        GB = max(GB, math.ceil(len(eb) / GP) or 1)

    G = 4 * (GA + GB)

    # Slot order is quarter-major (A-run then B-run inside each quarter) so
    # each quarter is ONE contiguous PE accumulation group (start..stop) in
    # its PSUM bank region — accumulation groups must not interleave.
    # Gather calls: one per (quarter, phase), issued in slot order; queues
    # assigned so every queue moves identical bytes per window.
    calls = []  # (g0, g1, isA, queue)
    queue_map = [0, 2, 1, 3, 2, 0, 3, 1]
    for q in range(4):
        base = q * (GA + GB)
        calls.append((base, base + GA, True, queue_map[2 * q]))
        calls.append((base + GA, base + GA + GB, False, queue_map[2 * q + 1]))

    # slot -> (phase, quarter, j)
    def slot_info(g):
        q, r = divmod(g, GA + GB)
        if r < GA:
            return 0, q, r
        return 1, q, r - GA

    per_core = []
    block_map = np.full((N_CORES, NW, 4), -1, dtype=np.int64)
    for k in range(N_CORES):
        wt = np.zeros((NW, GP, G, D), dtype=BF16)
        # pad gather slots are -1 (descriptor skipped via num_idxs_reg);
        # window 0 stays fully gathered (idx 0 pads) so every fj buffer slot
        # is initialized with finite data before any skipped-slot reuse.
        itf = np.full((NW, G * GP), -1, dtype=np.int16)
        itf[0] = 0
        ct = np.zeros((GP, NW, G), dtype=BF16)
        ncnt = np.zeros((NW, len(calls)), dtype=np.int32)
        for wi in range(NW):
            for q in range(4):
                t = cuts[k] + 4 * wi + q
                if t >= cuts[k + 1]:
                    continue
                block_map[k, wi, q] = t
                ea, eb = per_block[t]
                base = q * (GA + GB)
                for p, (el, gq, g0) in enumerate(
                    ((ea, GA, base), (eb, GB, base + GA))
                ):
                    n = len(el)
                    ncnt[wi, 2 * q + p] = n
                    if n == 0:
                        continue
                    j = np.arange(n)
                    gs = g0 + j // GP
                    lane = j % GP
                    wt[wi, lane, gs, :] = w[el].astype(BF16)
                    ct[lane, wi, gs] = (seg[el] - GP * t).astype(BF16)
                    itf[wi, gs * GP + lane] = (
                        idx[el] - (0 if p == 0 else TBL_SPLIT)
                    ).astype(np.int16)
        # per-call true counts: >=1 valid idx required; window 0 is full
        for wi in range(NW):
            for ci, (g0, g1, _isA, _q) in enumerate(calls):
                if wi == 0:
                    ncnt[wi, ci] = (g1 - g0) * GP
                elif ncnt[wi, ci] == 0:
                    itf[wi, g0 * GP] = 0
                    ncnt[wi, ci] = 1
        it = itf.reshape(NW, G * GP // 16, 16).transpose(2, 0, 1)  # [16, NW, s]
        it = np.tile(it, (8, 1, 1))
        ncnt_t = np.broadcast_to(ncnt[None, :, :], (128, NW, len(calls)))
        per_core.append(
            dict(
                wt=np.ascontiguousarray(wt),
                it=np.ascontiguousarray(it),
                ct=np.ascontiguousarray(ct),
                ncnt=np.ascontiguousarray(ncnt_t.astype(np.int32)),
            )
        )

    xT = np.zeros((D, NAP), dtype=BF16)
    xT[:, :N_ATOMS] = x.T.astype(BF16)
    iota = np.broadcast_to(np.arange(GP, dtype=np.float32), (GP, GP)).astype(BF16)
    shared = dict(
        xT=np.ascontiguousarray(xT),
        W1=np.ascontiguousarray(W1.astype(BF16)),
        W2=np.ascontiguousarray(np.asarray(W2, np.float32)),
        iota=np.ascontiguousarray(iota),
    )
    plan = dict(
        NW=NW, GA=GA, GB=GB, G=G, calls=calls,
        cuts=cuts, block_map=block_map, slot_info=[slot_info(g) for g in range(G)],
    )
    return per_core, shared, plan


def emulate_device(per_core, shared, plan):
    """Numpy mirror of the device program (bf16 rounding included)."""
    NW, G, GA, GB = plan["NW"], plan["G"], plan["GA"], plan["GB"]
    xT = shared["xT"].astype(np.float32)
    W1 = shared["W1"].astype(np.float32)
    W2 = shared["W2"].astype(np.float32)
    f = (xT.T @ W1).astype(BF16)  # [NAP, D]
    outs = []
    for k in range(N_CORES):
        m = per_core[k]
        out = np.zeros((NW, D, 4 * GP), dtype=np.float32)
        for wi in range(NW):
            idxw = (
                m["it"][:16, wi, :].T.reshape(-1).astype(np.int64)
            )  # [G*128] in (s p) order
            conv = np.zeros((D, 4, GP), dtype=np.float32)
            for g in range(G):
                p, q, j = plan["slot_info"][g]
                lanes = np.arange(GP)
                li = idxw[g * GP + lanes] + (0 if p == 0 else TBL_SPLIT)
                fj = f[li].astype(np.float32)  # [128, D]
                wf = (m["wt"][wi, :, g, :].astype(np.float32) * fj).astype(BF16)
                c = m["ct"][:, wi, g].astype(np.int64)
                S = (c[:, None] == np.arange(GP)[None, :]).astype(np.float32)
                conv[:, q, :] += wf.astype(np.float32).T @ S
            cvs = conv.reshape(D, 4 * GP)
            out[wi] = W2.T @ cvs
        outs.append(out)
    return outs


def host_combine(outs, plan, b2):
    bm = plan["block_map"]
    full = np.zeros((NAP, D), dtype=np.float32)
    for k in range(N_CORES):
        for wi in range(plan["NW"]):
            for q in range(4):
                t = bm[k, wi, q]
                if t < 0:
                    continue
                full[t * GP : (t + 1) * GP, :] = outs[k][wi][:, q * GP : (q + 1) * GP].T
    return full[:N_ATOMS] + np.asarray(b2, np.float32)[None, :]


def build_program(plan, reps=1, variant="full"):
    import concourse.bacc as bacc
    import concourse.mybir as mybir
    import concourse.tile as tile

    fp32 = mybir.dt.float32
    bf16 = mybir.dt.bfloat16
    i16 = mybir.dt.int16

    NW, G, GA, GB = plan["NW"], plan["G"], plan["GA"], plan["GB"]
    calls = plan["calls"]
    slot_info = plan["slot_info"]

    nc = bacc.Bacc(
        "TRN2",
        target_bir_lowering=False,
        debug=False,
        num_devices=N_CORES,
        num_swdge_queues=4,
    )

    xT_d = nc.dram_tensor("xT", [D, NAP], bf16, kind="ExternalInput")
    W1_d = nc.dram_tensor("W1", [D, D], bf16, kind="ExternalInput")
    W2_d = nc.dram_tensor("W2", [D, D], fp32, kind="ExternalInput")
    iota_d = nc.dram_tensor("iota", [GP, GP], bf16, kind="ExternalInput")
    wt_d = nc.dram_tensor("wt", [NW, GP, G, D], bf16, kind="ExternalInput")
    it_d = nc.dram_tensor("it", [128, NW, G * GP // 16], i16, kind="ExternalInput")
    ct_d = nc.dram_tensor("ct", [GP, NW, G], bf16, kind="ExternalInput")
    ncnt_d = nc.dram_tensor(
        "ncnt", [128, NW, len(calls)], mybir.dt.int32, kind="ExternalInput"
    )
    out_d = nc.dram_tensor("out", [NW, D, 4 * GP], fp32, kind="ExternalOutput")

    with tile.TileContext(nc) as tc:
        with (
            tc.tile_pool(name="consts", bufs=1) as consts,
            tc.tile_pool(name="dram", bufs=1, space="DRAM") as dram_pool,
        ):
            # double-buffered so rep r+1's f-phase overlaps rep r's gathers
            fA_ds = [
                dram_pool.tile([TBL_SPLIT, D], bf16, name=f"fA{i}") for i in range(2)
            ]
            fB_ds = [
                dram_pool.tile([NAP - TBL_SPLIT, D], bf16, name=f"fB{i}")
                for i in range(2)
            ]

            W1_sb = consts.tile([D, D], bf16)
            nc.sync.dma_start(W1_sb[:], W1_d[:])
            W2_sb = consts.tile([D, D], fp32)
            nc.sync.dma_start(W2_sb[:], W2_d[:])
            iota_sb = consts.tile([GP, GP], bf16)
            nc.sync.dma_start(iota_sb[:], iota_d[:])
            it_sb = consts.tile([128, NW, G * GP // 16], i16)
            nc.sync.dma_start(it_sb[:], it_d[:])
            ct_sb = consts.tile([GP, NW, G], bf16)
            nc.sync.dma_start(ct_sb[:], ct_d[:])
            ncnt_sb = consts.tile([128, NW, len(calls)], mybir.dt.int32)
            nc.sync.dma_start(ncnt_sb[:], ncnt_d[:])
            gcnt = nc.gpsimd.alloc_register("gcnt")

            for _rep in range(reps):
                fA_d = fA_ds[_rep % 2]
                fB_d = fB_ds[_rep % 2]
                # ---------------- f-phase: f = x @ W1 ----------------
                CH = 4
                nchunks = NAP // (CH * GP)
                with (
                    tc.tile_pool(name="xt", bufs=3) as xt_pool,
                    tc.tile_pool(name="fsb", bufs=3) as fsb_pool,
                    tc.tile_pool(name="fps", bufs=4, space="PSUM") as fps_pool,
                ):
                    for ci in range(nchunks):
                        a0 = ci * CH * GP
                        xt = xt_pool.tile([D, CH * GP], bf16)
                        nc.sync.dma_start(xt[:], xT_d[:, a0 : a0 + CH * GP])
                        fsb = fsb_pool.tile([GP, CH, D], bf16)
                        fps = fps_pool.tile([GP, CH, D], fp32)
                        for i in range(CH):
                            nc.tensor.matmul(
                                fps[:, i, :],
                                xt[:, i * GP : (i + 1) * GP],
                                W1_sb[:],
                                start=True,
                                stop=True,
                            )
                        nc.scalar.copy(fsb[:], fps[:])
                        if a0 < TBL_SPLIT:
                            dst = fA_d[a0 : a0 + CH * GP, :]
                        else:
                            dst = fB_d[a0 - TBL_SPLIT : a0 - TBL_SPLIT + CH * GP, :]
                        nc.sync.dma_start(
                            dst.rearrange("(i p) d -> p i d", p=GP),
                            fsb[:],
                        )

                # ---------------- main loop ----------------
                with (
                    tc.tile_pool(name="wsb", bufs=4) as w_pool,
                    tc.tile_pool(name="fj", bufs=4) as fj_pool,
                    tc.tile_pool(name="S", bufs=4) as s_pool,
                    tc.tile_pool(name="cvs", bufs=2) as cvs_pool,
                    tc.tile_pool(name="os", bufs=2) as os_pool,
                    tc.tile_pool(name="cvp", bufs=2, space="PSUM") as cvp_pool,
                    tc.tile_pool(name="otp", bufs=2, space="PSUM") as otp_pool,
                ):
                    for wi in range(NW):
                        cvp = cvp_pool.tile([D, 4, GP], fp32)
                        for ci, (g0, g1, isA, qnum) in enumerate(calls):
                            cs = g1 - g0
                            w_sb = w_pool.tile([GP, CHUNK_MAX, D], bf16, tag="wsb")
                            nc.sync.dma_start(
                                w_sb[:, :cs, :], wt_d[wi][:, g0:g1, :]
                            )
                            fj = fj_pool.tile([GP, CHUNK_MAX, D], bf16, tag="fj")
                            tblap = fA_d[:] if isA else fB_d[:]
                            if variant == "nogather":
                                nc.vector.tensor_copy(fj[:, :cs, :], w_sb[:, :cs, :])
                            else:
                                nc.gpsimd.reg_load(
                                    gcnt, ncnt_sb[0:1, wi, ci : ci + 1]
                                )
                                nc.gpsimd.dma_gather(
                                    fj[:, :cs, :],
                                    tblap,
                                    it_sb[:, wi, g0 * 8 : g1 * 8],
                                    cs * GP,
                                    gcnt,
                                    D,
                                    single_packet=False,
                                    queue_num=qnum,
                                )
                            nc.vector.tensor_mul(
                                fj[:, :cs, :], fj[:, :cs, :], w_sb[:, :cs, :]
                            )
                            s_sb = s_pool.tile([GP, CHUNK_MAX, D], bf16, tag="S")
                            nc.vector.tensor_tensor(
                                s_sb[:, :cs, :],
                                iota_sb[:].unsqueeze(1).broadcast_to((GP, cs, GP)),
                                ct_sb[:, wi, g0:g1]
                                .unsqueeze(2)
                                .broadcast_to((GP, cs, GP)),
                                mybir.AluOpType.is_equal,
                            )
                            for j in range(cs):
                                g = g0 + j
                                p, q, jq = slot_info[g]
                                nc.tensor.matmul(
                                    cvp[:, q, :],
                                    fj[:, j, :],
                                    s_sb[:, j, :],
                                    start=(p == 0 and jq == 0),
                                    stop=(p == 1 and jq == GB - 1),
                                )
                        cvs = cvs_pool.tile([D, 4 * GP], fp32)
                        nc.scalar.copy(cvs[:], cvp[:].rearrange("d w a -> d (w a)"))
                        otp = otp_pool.tile([D, 4 * GP], fp32)
                        nc.tensor.matmul(
                            otp[:], W2_sb[:], cvs[:], start=True, stop=True
                        )
                        osb = os_pool.tile([D, 4 * GP], fp32)
                        nc.scalar.copy(osb[:], otp[:])
                        nc.sync.dma_start(out_d[wi], osb[:])

    nc.compile()
    return nc


# ---------------------------------------------------------------------------
# execution helpers
# ---------------------------------------------------------------------------


def _exec_setup(nc, in_maps):
    import jax
    from jax.sharding import Mesh, PartitionSpec, NamedSharding
    from jax.experimental.shard_map import shard_map
    from concourse.bass2jax import (
        _bass_exec_p,
        install_neuronx_cc_hook,
        partition_id_tensor,
    )
    import concourse.mybir as mybir

    install_neuronx_cc_hook()
    partition_name = nc.partition_id_tensor.name if nc.partition_id_tensor else None

    in_names, out_names, out_avals, zero_outs = [], [], [], []
    for alloc in nc.m.functions[0].allocations:
        if not isinstance(alloc, mybir.MemoryLocationSet):
            continue
        name = alloc.memorylocations[0].name
        if alloc.kind == "ExternalInput":
            if name != partition_name:
                in_names.append(name)
        elif alloc.kind == "ExternalOutput":
            out_names.append(name)
            dt = mybir.dt.np(alloc.dtype)
            out_avals.append(jax.core.ShapedArray(tuple(alloc.tensor_shape), dt))
            zero_outs.append(np.zeros(tuple(alloc.tensor_shape), dt))
    all_names = in_names + out_names
    if partition_name is not None:
        all_names = all_names + [partition_name]

    def _body(*args):
        operands = list(args)
        if partition_name is not None:
            operands.append(partition_id_tensor())
        outs = _bass_exec_p.bind(
            *operands,
            out_avals=tuple(out_avals),
            in_names=tuple(all_names),
            out_names=tuple(out_names),
            lowering_input_output_aliases=(),
            sim_require_finite=True,
            sim_require_nnan=True,
            nc=nc,
        )
        return tuple(outs)

    devices = jax.devices()[:N_CORES]
    mesh = Mesh(np.asarray(devices), ("core",))
    nin = len(in_names) + len(zero_outs)
    sharded = jax.jit(
        shard_map(
            _body,
            mesh=mesh,
            in_specs=(PartitionSpec("core"),) * nin,
            out_specs=(PartitionSpec("core"),) * len(out_names),
            check_rep=False,
        ),
        keep_unused=True,
    )
    concat = [
        np.concatenate([np.asarray(in_maps[c][n]) for c in range(N_CORES)], axis=0)
        for n in in_names
    ] + [
        np.zeros((N_CORES * z.shape[0], *z.shape[1:]), z.dtype) for z in zero_outs
    ]
    sh = NamedSharding(mesh, PartitionSpec("core"))
    dev_in = [jax.device_put(a, sh) for a in concat]
    return sharded, dev_in, out_avals, out_names


def make_in_maps(per_core, shared):
    out = []
    for k in range(N_CORES):
        m = dict(shared)
        m.update(per_core[k])
        out.append(m)
    return out


def run_program(nc, per_core, shared):
    import jax

    sharded, dev_in, out_avals, out_names = _exec_setup(
        nc, make_in_maps(per_core, shared)
    )
    out = sharded(*dev_in)
    jax.block_until_ready(out)
    i = out_names.index("out")
    arr = np.asarray(out[i]).reshape(N_CORES, *out_avals[i].shape)
    return [arr[c] for c in range(N_CORES)]


def bench_program(nc, per_core, shared, iters=24):
    import jax

    sharded, dev_in, out_avals, out_names = _exec_setup(
        nc, make_in_maps(per_core, shared)
    )
    out = sharded(*dev_in)
    jax.block_until_ready(out)
    batch = 8
    times = []
    for _ in range(max(1, iters // batch)):
        tb = time.perf_counter()
        outs = [sharded(*dev_in) for _ in range(batch)]
        jax.block_until_ready(outs)
        times.append((time.perf_counter() - tb) / batch)
    i = out_names.index("out")
    arr = np.asarray(out[i]).reshape(N_CORES, *out_avals[i].shape)
    return min(times), [arr[c] for c in range(N_CORES)]


def bench_pair(nc_lo, nc_hi, per_core, shared, rounds=6, batch=8):
    """Interleaved steady-state timing of two compiled programs.

    Returns (min_lo, min_hi) per-execution wall times. Interleaving the
    rounds makes the difference robust against slow drift in the fixed
    per-dispatch overhead of the tunneled PJRT path.
    """
    import jax

    in_maps = make_in_maps(per_core, shared)
    lo = _exec_setup(nc_lo, in_maps)
    hi = _exec_setup(nc_hi, in_maps)
    for sharded, dev_in, _, _ in (lo, hi):
        out = sharded(*dev_in)
        jax.block_until_ready(out)
    t_lo, t_hi = [], []
    for _ in range(rounds):
        for sharded, dev_in, acc in ((lo[0], lo[1], t_lo), (hi[0], hi[1], t_hi)):
            tb = time.perf_counter()
            outs = [sharded(*dev_in) for _ in range(batch)]
            jax.block_until_ready(outs)
            acc.append((time.perf_counter() - tb) / batch)
    return min(t_lo), min(t_hi)


def kernel(x, w_ij, seg_i, idx_j, seg_i_sum, W1, W2, b2, _emulate=False):
    per_core, shared, plan = prep_inputs(x, w_ij, seg_i, idx_j, W1, W2)
    if _emulate:
        outs = emulate_device(per_core, shared, plan)
    else:
        nc = build_program(plan, reps=1)
        outs = run_program(nc, per_core, shared)
    return host_combine(outs, plan, b2)
